# revision 35
# baseline (speedup 1.0000x reference)
"""Trainium2 Bass kernel for CrossAttentionFusion (B=4, S=2048, 768/512->1024).

Sharding: 8 cores = 4 batches x 2 row-halves, fully SPMD, no collectives.
Per-core inputs are row-rotated so the core's 1024 output rows are always
rows 0:1023 (attention is permutation-equivariant under a joint row permute
of semantic/graph).

Per core:
  P1  sem = LN(semantic @ W_sp + b_sp)   (all 2048 rows)   -> sem_nat (DRAM),
      semT (rows 0:1023 in SBUF, rows 1024:2047 in DRAM; PE-transposed)
  P2  gra likewise -> gra_nat, graT
  P3  for each 512-col chunk of t in [0,1024):
        E[:, chunk] = exp(attn/32)  (16x8 f32r matmuls), colsum via ones-matmul,
        gra_ctx^T[d, chunk] = sum_s sem[s,d] E[s,chunk]  (psum accumulate), then
        * recip(colsum) + graT  -> fusedT_lo (DRAM)
  P4  same transposed: E^T[t, s-chunk], rowsum, sem_ctx^T * recip(rowsum)+semT
      -> fusedT_up (SBUF)
  P5  out = relu(fusedT.T @ W_out + b_out)  rows 0:1023

All matmuls run as float32r (TF32-like, 1 cycle/row on the PE at free-dim 512,
fp32 storage).  exp/softmax needs no max subtraction: |logit| <= 32 since LN
rows have norm sqrt(D)=32, and exp(32) is far below f32 overflow.
"""

import numpy as np
from contextlib import ExitStack

try:
    import concourse.bass as bass
except ImportError:  # grading env may not have concourse on sys.path
    import sys

    sys.path.insert(0, "/opt/trn_rl_repo")
    import concourse.bass as bass

import concourse.mybir as mybir
import concourse.tile as tile
from concourse.bass_utils import run_bass_kernel_spmd
from concourse.masks import make_identity

P = 128
S = 2048
SA = 1024  # rows per core
KSEM = 768  # 6 k-tiles
KGRA = 512  # 4 k-tiles
D = 1024  # 8 d-tiles
C2 = 2048  # out-proj contraction
NT = S // P  # 16
NTA = SA // P  # 8
EPS = 1e-5
ISCALE = 1.0 / 32.0

F32 = mybir.dt.float32
F32R = mybir.dt.float32r
AF = mybir.ActivationFunctionType
ALU = mybir.AluOpType


def _r(ap):
    return ap.bitcast(F32R)


def build(flags=()):
    """Build the SPMD single-core program.  flags: subset of
    {"b_sp","b_gp","g1","be1","g2","be2","b_out"} enabling the generic
    (non-trivial bias/gamma/beta) paths."""
    flags = set(flags)
    nc = bass.Bass(trn_type="TRN2", target_bir_lowering=False, debug=False)

    semantic = nc.dram_tensor("semantic", [S, KSEM], F32, kind="ExternalInput").ap()
    graph = nc.dram_tensor("graph", [S, KGRA], F32, kind="ExternalInput").ap()
    w_sp = nc.dram_tensor("w_sp", [KSEM, D], F32, kind="ExternalInput").ap()
    w_gp = nc.dram_tensor("w_gp", [KGRA, D], F32, kind="ExternalInput").ap()
    w_out = nc.dram_tensor("w_out", [C2, D], F32, kind="ExternalInput").ap()
    vecs = {
        n: nc.dram_tensor(n, [1, D], F32, kind="ExternalInput").ap()
        for n in ("b_sp", "b_gp", "g1", "be1", "g2", "be2", "b_out")
    }
    ones_d = nc.dram_tensor("ones", [P, 1], F32, kind="ExternalInput").ap()
    out = nc.dram_tensor("out", [SA, D], F32, kind="ExternalOutput").ap()

    sem_nat = nc.dram_tensor("sem_nat_scr", [P, NT, D], F32R, kind="Internal").ap()
    gra_nat = nc.dram_tensor("gra_nat_scr", [P, NT, D], F32R, kind="Internal").ap()
    semT_Bd = nc.dram_tensor("semT_B_scr", [P, 8, SA], F32R, kind="Internal").ap()
    graT_Bd = nc.dram_tensor("graT_B_scr", [P, 8, SA], F32R, kind="Internal").ap()
    flo_d = nc.dram_tensor("fusedT_lo_scr", [P, NTA, SA], F32R, kind="Internal").ap()

    with tile.TileContext(nc) as tc:
        const_cm = tc.tile_pool(name="const", bufs=1)
        const = const_cm.__enter__()
        identity = const.tile([P, P], F32)
        make_identity(nc, identity)
        # DMA (not memset) so the producer carries the f32r dtype tag the
        # BIR verifier requires on fp32r-matmul operands; memset can't
        # encode an f32r immediate.
        ones_col = const.tile([P, 1], F32R)
        nc.sync.dma_start(out=ones_col, in_=ones_d.bitcast(F32R))
        ones_row = const.tile([1, P], F32)
        nc.vector.memset(ones_row, 1.0)
        eps_t = const.tile([P, 1], F32)
        nc.vector.memset(eps_t, EPS)

        def bcast_row(row_ap):  # [1, D] dram -> [P, D] sbuf (replicated rows)
            t = const.tile([P, D], F32, name="bc_" + row_ap.tensor.name)
            nc.sync.dma_start(out=t, in_=row_ap.to_broadcast((P, D)))
            return t

        b_sp_t = bcast_row(vecs["b_sp"]) if "b_sp" in flags else None
        b_gp_t = bcast_row(vecs["b_gp"]) if "b_gp" in flags else None
        g1_t = bcast_row(vecs["g1"]) if "g1" in flags else None
        be1_t = bcast_row(vecs["be1"]) if "be1" in flags else None
        g2_t = bcast_row(vecs["g2"]) if "g2" in flags else None
        be2_t = bcast_row(vecs["be2"]) if "be2" in flags else None
        b_out_t = bcast_row(vecs["b_out"]) if "b_out" in flags else None

        # fusedT upper half lives from the attention phase through out-proj;
        # open it early so pool releases stay LIFO.
        fup_cm = tc.tile_pool(name="fup", bufs=1)
        fusedT_up = fup_cm.__enter__().tile([P, NTA, SA], F32R, tag="fup")

        # Resident transposed activations, rows-of-interest only: [P, 8, 1024].
        semTA_cm = tc.tile_pool(name="semTA", bufs=1)
        semT_A = semTA_cm.__enter__().tile([P, 8, SA], F32R, tag="semT_A")

        def proj_phase(feat, w_dram, kt, bias_t, gamma_t, beta_t, TA, TBd, nat_dram):
            K = kt * P
            with ExitStack() as cx:
                wpool = cx.enter_context(tc.tile_pool(name="wproj", bufs=1))
                w_sb = wpool.tile([P, kt, D], F32R)
                nc.sync.dma_start(
                    out=w_sb,
                    in_=w_dram.rearrange("(ko p) n -> p ko n", p=P).bitcast(F32R),
                )
                inpool = cx.enter_context(tc.tile_pool(name="xin", bufs=3))
                xtp = cx.enter_context(tc.tile_pool(name="xT", bufs=3))
                opool = cx.enter_context(tc.tile_pool(name="xo", bufs=3))
                bpool = cx.enter_context(tc.tile_pool(name="tbounce", bufs=3))
                stat = cx.enter_context(tc.tile_pool(name="stat", bufs=4))
                ptx = cx.enter_context(tc.tile_pool(name="ptx", bufs=2, space="PSUM"))
                ppp = cx.enter_context(tc.tile_pool(name="ppp", bufs=2, space="PSUM"))
                feat_r = feat.rearrange("(n p) k -> p n k", p=P)
                for st in range(NT):
                    x_in = inpool.tile([P, K], F32, tag="x_in")
                    nc.sync.dma_start(out=x_in, in_=feat_r[:, st, :])
                    # transpose the input tile: [s, k] -> [k, s]
                    pt = ptx.tile([P, D], F32, tag="ptx")
                    for k in range(kt):
                        nc.tensor.transpose(
                            pt[:, k * P : (k + 1) * P],
                            x_in[:, k * P : (k + 1) * P],
                            identity,
                        )
                    xT = xtp.tile([P, kt, P], F32R, tag="xT")
                    nc.scalar.copy(
                        out=xT.rearrange("p a b -> p (a b)"), in_=pt[:, :K]
                    )
                    # projection matmuls, accumulate over k
                    pp = ppp.tile([P, D], F32, tag="pp")
                    for k in range(kt):
                        for nch in range(2):
                            nc.tensor.matmul(
                                pp[:, nch * 512 : (nch + 1) * 512],
                                lhsT=xT[:, k, :],
                                rhs=w_sb[:, k, nch * 512 : (nch + 1) * 512],
                                start=(k == 0),
                                stop=(k == kt - 1),
                            )
                    if bias_t is not None:
                        nc.vector.tensor_add(out=pp, in0=pp, in1=bias_t)
                    # layernorm over the free (d) axis
                    stats = stat.tile([P, 2, 6], F32, tag="bn")
                    for h in range(2):
                        nc.vector.bn_stats(
                            out=stats[:, h, :], in_=pp[:, h * 512 : (h + 1) * 512]
                        )
                    mv = stat.tile([P, 2], F32, tag="mv")
                    nc.vector.bn_aggr(out=mv, in_=stats)
                    rstd = stat.tile([P, 1], F32, tag="rstd")
                    nc.scalar.activation(
                        out=rstd, in_=mv[:, 1:2], func=AF.Sqrt, bias=eps_t
                    )
                    nc.vector.reciprocal(out=rstd, in_=rstd)
                    xo = opool.tile([P, D], F32, tag="xo")
                    nc.vector.tensor_scalar(
                        out=xo,
                        in0=pp,
                        scalar1=mv[:, 0:1],
                        scalar2=rstd,
                        op0=ALU.subtract,
                        op1=ALU.mult,
                    )
                    if gamma_t is not None:
                        nc.vector.tensor_tensor(out=xo, in0=xo, in1=gamma_t, op=ALU.mult)
                    if beta_t is not None:
                        nc.vector.tensor_tensor(out=xo, in0=xo, in1=beta_t, op=ALU.add)
                    nc.sync.dma_start(out=nat_dram[:, st, :], in_=xo.bitcast(F32R))
                    # transpose the normalized tile into semT/graT (A resident,
                    # B staged to DRAM via an SBUF bounce)
                    pt2 = ptx.tile([P, D], F32, tag="ptx")
                    for dd in range(8):
                        nc.tensor.transpose(
                            pt2[:, dd * P : (dd + 1) * P],
                            xo[:, dd * P : (dd + 1) * P],
                            identity,
                        )
                    if st < NTA:
                        nc.vector.tensor_copy(
                            out=TA[:, :, st * P : (st + 1) * P],
                            in_=pt2.rearrange("p (a b) -> p a b", a=8),
                        )
                    else:
                        col = st - NTA
                        tb = bpool.tile([P, 8, P], F32R, tag="tb")
                        nc.vector.tensor_copy(
                            out=tb, in_=pt2.rearrange("p (a b) -> p a b", a=8)
                        )
                        nc.sync.dma_start(
                            out=TBd[:, :, col * P : (col + 1) * P], in_=tb
                        )

        proj_phase(semantic, w_sp, 6, b_sp_t, g1_t, be1_t, semT_A, semT_Bd, sem_nat)

        graTA_cm = tc.tile_pool(name="graTA", bufs=1)
        graT_A = graTA_cm.__enter__().tile([P, 8, SA], F32R, tag="graT_A")

        proj_phase(graph, w_gp, 4, b_gp_t, g2_t, be2_t, graT_A, graT_Bd, gra_nat)

        small_cm = tc.tile_pool(name="small", bufs=1)
        small = small_cm.__enter__()
        eh_cm = tc.tile_pool(name="ehalf", bufs=1)
        eh_pool = eh_cm.__enter__()

        def attn_ctx_phase(lT_A, lT_Bd, rT_A, ctx_nat, add_T, variant):
            """variant 0: E_col + gra_ctx -> flo_d;  variant 1: ET + sem_ctx -> fusedT_up."""
            for hch in range(2):
                hsl = slice(hch * 512, (hch + 1) * 512)
                ehalf = eh_pool.tile([P, NT, 512], F32R, tag="ehalf")
                with ExitStack() as c1:
                    lpool = c1.enter_context(tc.tile_pool(name="lstream", bufs=3))
                    pe_pool = c1.enter_context(
                        tc.tile_pool(name="pe", bufs=2, space="PSUM")
                    )
                    pcs_pool = c1.enter_context(
                        tc.tile_pool(name="pcs", bufs=1, space="PSUM")
                    )
                    pcs = pcs_pool.tile([1, 512], F32, tag="pcs")
                    for it in range(NT):
                        if it < NTA:
                            lT = lT_A[:, :, it * P : (it + 1) * P]
                        else:
                            col = it - NTA
                            lT = lpool.tile([P, 8, P], F32R, tag="lT")
                            nc.sync.dma_start(
                                out=lT, in_=lT_Bd[:, :, col * P : (col + 1) * P]
                            )
                        pe = pe_pool.tile([P, 512], F32, tag="pe")
                        for k in range(8):
                            nc.tensor.matmul(
                                pe,
                                lhsT=lT[:, k, :],
                                rhs=rT_A[:, k, hsl],
                                start=(k == 0),
                                stop=(k == 7),
                            )
                        nc.scalar.activation(
                            out=ehalf[:, it, :], in_=pe, func=AF.Exp, scale=ISCALE
                        )
                        nc.tensor.matmul(
                            pcs,
                            lhsT=ones_col,
                            rhs=ehalf[:, it, :],
                            start=(it == 0),
                            stop=(it == NT - 1),
                        )
                    cs_sb = small.tile([1, 512], F32, tag="cs")
                    nc.scalar.copy(out=cs_sb, in_=pcs)
                    rec = small.tile([1, 512], F32, tag="rec")
                    nc.vector.reciprocal(out=rec, in_=cs_sb)
                    # broadcast rec across partitions via PE outer product
                    pbc = pcs_pool.tile([P, 512], F32, tag="pbc")
                    nc.tensor.matmul(
                        pbc, lhsT=ones_row, rhs=rec, start=True, stop=True
                    )
                    rec_b = small.tile([P, 512], F32, tag="rec_b")
                    nc.vector.tensor_copy(out=rec_b, in_=pbc)
                with ExitStack() as c2:
                    pg_pool = c2.enter_context(
                        tc.tile_pool(name="pg", bufs=8, space="PSUM")
                    )
                    spool = c2.enter_context(tc.tile_pool(name="ctx_in", bufs=3))
                    oopool = c2.enter_context(tc.tile_pool(name="ctx_o", bufs=2))
                    pgs = [
                        pg_pool.tile([P, 512], F32, tag="pg", name=f"pg{i}")
                        for i in range(8)
                    ]
                    for it in range(NT):
                        xt = spool.tile([P, D], F32R, tag="ctx_x")
                        nc.sync.dma_start(out=xt, in_=ctx_nat[:, it, :])
                        for dt in range(8):
                            nc.tensor.matmul(
                                pgs[dt],
                                lhsT=xt[:, dt * P : (dt + 1) * P],
                                rhs=ehalf[:, it, :],
                                start=(it == 0),
                                stop=(it == NT - 1),
                            )
                    for dt in range(8):
                        tmp = oopool.tile([P, 512], F32, tag="ctx_tmp")
                        nc.vector.tensor_tensor(
                            out=tmp, in0=pgs[dt], in1=rec_b, op=ALU.mult
                        )
                        if variant == 0:
                            tmp2 = oopool.tile([P, 512], F32R, tag="ctx_tmp2")
                            nc.vector.tensor_tensor(
                                out=tmp2, in0=tmp, in1=add_T[:, dt, hsl], op=ALU.add
                            )
                            nc.sync.dma_start(out=flo_d[:, dt, hsl], in_=tmp2)
                        else:
                            nc.vector.tensor_tensor(
                                out=fusedT_up[:, dt, hsl],
                                in0=tmp,
                                in1=add_T[:, dt, hsl],
                                op=ALU.add,
                            )

        # variant 0: E[:, t in A] + gra_ctx^T  (lhsT over all s = semT, rhs graT_A)
        attn_ctx_phase(semT_A, semT_Bd, graT_A, sem_nat, graT_A, 0)

        # variant 1: E^T[t, s in A] + sem_ctx^T (lhsT over all t = graT, rhs semT_A)
        attn_ctx_phase(graT_A, graT_Bd, semT_A, gra_nat, semT_A, 1)

        eh_cm.__exit__(None, None, None)
        small_cm.__exit__(None, None, None)
        graTA_cm.__exit__(None, None, None)
        semTA_cm.__exit__(None, None, None)

        # out projection
        with ExitStack() as c3:
            wpool = c3.enter_context(tc.tile_pool(name="wout", bufs=1))
            w_sb = wpool.tile([P, 16, D], F32R)
            nc.sync.dma_start(
                out=w_sb,
                in_=w_out.rearrange("(ko p) n -> p ko n", p=P).bitcast(F32R),
            )
            flo_pool = c3.enter_context(tc.tile_pool(name="flo", bufs=1))
            flo_sb = flo_pool.tile([P, NTA, SA], F32R)
            nc.sync.dma_start(out=flo_sb, in_=flo_d)
            po_pool = c3.enter_context(tc.tile_pool(name="po", bufs=2, space="PSUM"))
            oo_pool = c3.enter_context(tc.tile_pool(name="oo", bufs=3))
            out_r = out.rearrange("(n p) d -> p n d", p=P)
            for st in range(NTA):
                po = po_pool.tile([P, D], F32, tag="po")
                for kt in range(16):
                    lhs = (
                        fusedT_up[:, kt, st * P : (st + 1) * P]
                        if kt < 8
                        else flo_sb[:, kt - 8, st * P : (st + 1) * P]
                    )
                    for nch in range(2):
                        nc.tensor.matmul(
                            po[:, nch * 512 : (nch + 1) * 512],
                            lhsT=lhs,
                            rhs=w_sb[:, kt, nch * 512 : (nch + 1) * 512],
                            start=(kt == 0),
                            stop=(kt == 15),
                        )
                if b_out_t is not None:
                    nc.vector.tensor_add(out=po, in0=po, in1=b_out_t)
                oo = oo_pool.tile([P, D], F32, tag="oo")
                nc.scalar.activation(out=oo, in_=po, func=AF.Relu)
                nc.sync.dma_start(out=out_r[:, st, :], in_=oo)

        fup_cm.__exit__(None, None, None)
        const_cm.__exit__(None, None, None)

    return nc


def _split_multi_waits(bir_json: bytes) -> bytes:
    """walrus codegen allows only one sync-wait command per instruction for
    several ISA structs (LDW, DMA_DIRECT2D, ...), but the Tile scheduler
    attaches one wait per cross-engine producer.  Hoist all but the last
    wait of every instruction onto NoOps inserted just before it on the
    same engine (engines run block instructions in order, so the waits
    still all complete before the instruction issues)."""
    import json

    bir = json.loads(bir_json)
    fns = bir["functions"] if "functions" in bir else bir["modules"][0]["functions"]
    for fn in fns:
        blocks = fn["sb_blocks"] if "sb_blocks" in fn else fn["blocks"]
        for bb in blocks:
            out = []
            for inst in bb["instructions"]:
                si = inst.get("sync_info") or {}
                waits = si.get("on_wait") or []
                if len(waits) > 1:
                    for j, wt in enumerate(waits[:-1]):
                        out.append({
                            "name": f"{inst['name']}-w{j}",
                            "opcode": "NoOp",
                            "engine": inst["engine"],
                            "ins": [],
                            "outs": [],
                            "sync_info": {"on_update": [], "on_wait": [wt]},
                        })
                    inst["sync_info"] = {
                        "on_update": si.get("on_update") or [],
                        "on_wait": [waits[-1]],
                    }
                out.append(inst)
            bb["instructions"] = out
    return json.dumps(bir).encode()


def _install_wait_splitter():
    from concourse import bass2jax as _b2j
    from concourse.bass_utils import compile_bir_kernel as _orig

    if getattr(_b2j.compile_bir_kernel, "_wait_splitter", False):
        return

    def patched(bir_json, tmpdir, neff_name="file.neff"):
        return _orig(_split_multi_waits(bir_json), tmpdir, neff_name=neff_name)

    patched._wait_splitter = True
    _b2j.compile_bir_kernel = patched


_CACHE = {}


def _get_nc(flags):
    key = tuple(sorted(flags))
    if key not in _CACHE:
        _CACHE[key] = build(key)
    return _CACHE[key]


def make_in_maps(semantic_feature, graph_feature, W_sp, b_sp, W_gp, b_gp,
                 gamma1, beta1, gamma2, beta2, W_out, b_out):
    a = lambda x: np.ascontiguousarray(np.asarray(x, dtype=np.float32))
    sem_f, gra_f = a(semantic_feature), a(graph_feature)
    W_sp, W_gp, W_out = a(W_sp), a(W_gp), a(W_out)
    row = lambda x: a(x).reshape(1, D)
    vecs = {
        "b_sp": row(b_sp), "b_gp": row(b_gp), "g1": row(gamma1),
        "be1": row(beta1), "g2": row(gamma2), "be2": row(beta2),
        "b_out": row(b_out),
    }
    flags = set()
    for n in ("b_sp", "b_gp", "be1", "be2", "b_out"):
        if np.any(vecs[n]):
            flags.add(n)
    for n in ("g1", "g2"):
        if not np.all(vecs[n] == 1.0):
            flags.add(n)
    in_maps = []
    for c in range(8):
        b, h = divmod(c, 2)
        sem = sem_f[b]
        gra = gra_f[b]
        if h:
            sem = np.ascontiguousarray(np.concatenate([sem[SA:], sem[:SA]], 0))
            gra = np.ascontiguousarray(np.concatenate([gra[SA:], gra[:SA]], 0))
        m = {"semantic": sem, "graph": gra, "w_sp": W_sp, "w_gp": W_gp,
             "w_out": W_out, "ones": np.ones((P, 1), np.float32)}
        m.update(vecs)
        in_maps.append(m)
    return in_maps, flags


def kernel(semantic_feature, graph_feature, W_sp, b_sp, W_gp, b_gp,
           gamma1, beta1, gamma2, beta2, W_out, b_out, _trace=False):
    in_maps, flags = make_in_maps(
        semantic_feature, graph_feature, W_sp, b_sp, W_gp, b_gp,
        gamma1, beta1, gamma2, beta2, W_out, b_out)
    nc = _get_nc(flags)
    _install_wait_splitter()
    res = run_bass_kernel_spmd(nc, in_maps, core_ids=list(range(8)), trace=_trace)
    out = np.empty((4, S, D), np.float32)
    for c in range(8):
        b, h = divmod(c, 2)
        oc = res.results[c]["out"]
        if h:
            out[b, SA:] = oc[:]  # rotated rows 0:1024 == original rows 1024:2048
        else:
            out[b, :SA] = oc[:]
    kernel._last_results = res
    return out
